# revision 38
# baseline (speedup 1.0000x reference)
"""NetVLAD Trainium2 Bass kernel.

Full inputs in, full output out. Data-parallel over batch N=64 across 8
NeuronCores (8 samples per core); conv weight and centroids replicated.

Per-sample algorithm (mathematically equal to the reference, never
materializing the channel-normalized x):
  X = x[n]  [D=128, P=4800]  (D on SBUF partitions, contiguous in HBM)
  For each 128-wide chunk of P (p on partitions after a PE transpose):
    ss[p]    = sum_d X[d,p]^2
    inv_s    = ss^-0.5                   (DVE pow — keeps the ACT table
                                          set fixed: only Copy/Square/Exp)
    logitsT  = X_c^T @ Wt                (PE)
    e        = exp(logitsT * inv_s)      (softmax max-subtraction skipped:
                                          |logits*inv_s| <= ~1.2)
    sb       = e * (inv_s / Z),  Z = sum_k e
    acc     += [sb | e]^T @ [X_c^T | 1/Z]   (PE, PSUM accumulate)
  agg      = acc[0:64, 0:128];  sum_sa = acc[64:128, 128]
  vlad     = agg - sum_sa * centroids, then intra + global L2 norm.

Pipelining: per-chunk scalar work is batched into whole-sample ops
(one Square, one reduce, one Exp, ...); the accumulate matmuls of
sample n-2 are emitted between pass A of sample n so the PE never
waits on the scalar chain. The [sb|e] and [XT|1/Z] operands are bf16
(FWL fast weight load; f32 PSUM accumulation).
"""

import sys

if "/opt/trn_rl_repo" not in sys.path:
    sys.path.insert(0, "/opt/trn_rl_repo")

import numpy as np
from contextlib import ExitStack

N, D, HW, K = 64, 128, 4800, 64
NCORES = 8
NS = N // NCORES  # samples per core

CHUNKS = [(i * 128, min(128, HW - i * 128)) for i in range((HW + 127) // 128)]
NCH = len(CHUNKS)  # 38: 37 full + one 64-wide

_CACHE = {}


def _patch_act_tables():
    """Steer bacc's ACT table-set placement to the one set that covers
    every function we use (ln/exp/square/copy) so the kernel pays a single
    ACT_TABLE_LOAD instead of thrashing between per-anchor sets."""
    if _CACHE.get("act_patched"):
        return
    from concourse import bacc, mybir

    orig = bacc.get_activation_tables
    AF = mybir.ActivationFunctionType
    combo = "natural_log_exp_and_others"

    def patched(arch):
        t = {k: set(v) for k, v in orig(arch).items()}
        if combo in t:
            for name in t:
                if name != combo:
                    t[name] = t[name] - {AF.Ln, AF.Exp}
        return t

    bacc.get_activation_tables = patched
    _CACHE["act_patched"] = True


def _build_nc():
    import concourse.tile as tile
    from concourse import bacc, mybir

    _patch_act_tables()

    nc = bacc.Bacc(
        "TRN2",
        target_bir_lowering=False,
        debug=False,
        enable_asserts=False,
        num_devices=NCORES,
    )
    x_ap = nc.dram_tensor(
        "x", [NS, D, HW], mybir.dt.float32r, kind="ExternalInput"
    ).ap()
    wt_ap = nc.dram_tensor("wt", [D, K], mybir.dt.float32, kind="ExternalInput").ap()
    idw_ap = nc.dram_tensor(
        "idw", [D, 256], mybir.dt.float32r, kind="ExternalInput"
    ).ap()
    cent_ap = nc.dram_tensor(
        "cent", [K, D], mybir.dt.float32, kind="ExternalInput"
    ).ap()
    out_ap = nc.dram_tensor(
        "out", [NS, K, D], mybir.dt.float32, kind="ExternalOutput"
    ).ap()

    with tile.TileContext(nc) as tc:
        with ExitStack() as ctx:
            _body(ctx, tc, out_ap, x_ap, wt_ap, cent_ap, idw_ap)
    nc.compile()
    return nc


def _body(ctx, tc, out_ap, x_ap, wt_ap, cent_ap, idw_ap):
    import concourse.bass as bass
    from concourse import masks, mybir

    nc = tc.nc
    f32 = mybir.dt.float32
    bf16 = mybir.dt.bfloat16
    AF = mybir.ActivationFunctionType
    ALU = mybir.AluOpType
    X_AX = mybir.AxisListType.X

    singles = ctx.enter_context(tc.tile_pool(name="singles", bufs=1))
    xpool = ctx.enter_context(tc.tile_pool(name="xpool", bufs=3))
    xtrpool = ctx.enter_context(tc.tile_pool(name="xtrpool", bufs=5))
    ebpool = ctx.enter_context(tc.tile_pool(name="ebpool", bufs=2))
    sbtpool = ctx.enter_context(tc.tile_pool(name="sbtpool", bufs=2))
    lpool = ctx.enter_context(tc.tile_pool(name="lpool", bufs=3))
    scrpool = ctx.enter_context(tc.tile_pool(name="scrpool", bufs=2))
    smalls = ctx.enter_context(tc.tile_pool(name="smalls", bufs=6))
    tails = ctx.enter_context(tc.tile_pool(name="tails", bufs=1))
    pp_xt = ctx.enter_context(tc.tile_pool(name="pp_xt", bufs=2, space="PSUM"))
    pp_acc = ctx.enter_context(tc.tile_pool(name="pp_acc", bufs=1, space="PSUM"))
    pp_tiny = ctx.enter_context(tc.tile_pool(name="pp_tiny", bufs=1, space="PSUM"))

    def bcast(ap, n):
        # append a step-0 free dim: [..., n] broadcast view
        return bass.AP(tensor=ap.tensor, offset=ap.offset, ap=list(ap.ap) + [[0, n]])

    def mid_bcast(ap, n):
        # [p, f] -> [p, n, f] with step-0 middle dim
        return bass.AP(
            tensor=ap.tensor,
            offset=ap.offset,
            ap=[ap.ap[0], [0, n]] + list(ap.ap[1:]),
        )

    # constants
    f32r = mybir.dt.float32r
    ident = singles.tile([128, 128], f32)
    masks.make_identity(nc, ident[:])
    # fused rhs for pass A: [identity | Wt | zero-pad] — one matmul yields
    # [X_c^T | logits | junk]. Padded to 256 cols so the fp32r (single-pass
    # fp32) matmul streams at 1 cycle/row instead of fp32's 4. The tile is
    # declared float32r so every producer writes fp32r (BIR verifier rule).
    identwt = singles.tile([128, 256], f32r)
    nc.sync.dma_start(out=identwt[:], in_=idw_ap[:])
    cent_s = singles.tile([K, D], f32)
    nc.sync.dma_start(out=cent_s[:], in_=cent_ap[:])
    ones_col = singles.tile([K, 1], f32)
    nc.vector.memset(ones_col[:], 1.0)
    ones_row = singles.tile([1, K], f32)
    nc.vector.memset(ones_row[:], 1.0)

    GRP = 6  # fused-matmul chunks per PSUM tile (3 banks; 2 rotating bufs)
    groups = []
    c0 = 0
    while c0 < NCH:
        groups.append(list(range(c0, min(c0 + GRP, NCH))))
        c0 += GRP
    NG = len(groups)  # 7

    # pass-C interleave: chunk range emitted after group g of the round
    pc_slices = []
    base = 0
    for g in range(NG):
        take = (NCH - base + (NG - g) - 1) // (NG - g)
        pc_slices.append((base, base + take))
        base += take

    state = {}  # per-sample live tiles

    def emit_dma(n):
        xs = xpool.tile([D, HW], mybir.dt.float32r, tag="xs")
        if n == 0:
            # per-group pieces: group-0 matmuls start after ~1/7 of the load
            for grp in groups:
                p0 = CHUNKS[grp[0]][0]
                p1 = CHUNKS[grp[-1]][0] + CHUNKS[grp[-1]][1]
                nc.sync.dma_start(out=xs[:, p0:p1], in_=x_ap[n, :, p0:p1])
        else:
            nc.sync.dma_start(out=xs[:, 0 : HW // 2], in_=x_ap[n, :, 0 : HW // 2])
            nc.sync.dma_start(out=xs[:, HW // 2 :], in_=x_ap[n, :, HW // 2 :])
        state.setdefault(n, {})["xs"] = xs

    def alloc_sample(n):
        st = state.setdefault(n, {})
        # [XT | s] per chunk (bf16): cols 0:128 = X_c^T, col 128 = ||x_p||
        st["xtr"] = xtrpool.tile([128, NCH, 129], bf16, tag="xtr", name="xtr")
        # raw logits [p, c, k]
        st["lgs"] = lpool.tile([128, NCH, K], bf16, tag="lgs", name="lgs")

    def emit_passA_group(n, g):
        st = state[n]
        xs = st["xs"]
        grp = groups[g]
        xt_p = pp_xt.tile([128, GRP, 256], f32, tag="xt")
        for j, c in enumerate(grp):
            p0, w = CHUNKS[c]
            nc.tensor.matmul(
                xt_p[:w, j, 0:256],
                lhsT=xs[:, p0 : p0 + w],
                rhs=identwt[:],
                start=True,
                stop=True,
            )
        return xt_p, grp[0], len(grp)

    def emit_evacs(n, g, xt_p, gc, gn):
        st = state[n]
        xtr, lgs = st["xtr"], st["lgs"]
        # X^T columns -> SBUF bf16 (one consolidated ACT copy per tile)
        nc.scalar.copy(xtr[:, gc : gc + gn, 0:128], xt_p[:, 0:gn, 0:128])
        # logits -> lgs [p, c, k]; two tiles' worth on DVE, rest on ACT
        if g in (1, 3):
            nc.vector.tensor_copy(lgs[:, gc : gc + gn, :], xt_p[:, 0:gn, 128:192])
        else:
            nc.scalar.copy(lgs[:, gc : gc + gn, :], xt_p[:, 0:gn, 128:192])

    # ---- stage B: ss = rowsum(X_c^T^2), inv_s, s-col (sample lag 1) ----
    def st_sq(n):
        st = state[n]
        x2t = scrpool.tile([128, NCH, 128], bf16, tag="x2t", bufs=1)
        st["x2t"] = x2t
        xv = st["xtr"][:, :, 0:128]
        nc.vector.tensor_tensor(out=x2t[:], in0=xv, in1=xv, op=ALU.mult)

    def st_tree12(n):
        st = state[n]
        x2t = st["x2t"]
        t1 = scrpool.tile([128, NCH, 64], bf16, tag="t1", bufs=2)
        t2 = scrpool.tile([128, NCH, 32], bf16, tag="t2", bufs=2)
        st["t2"] = t2
        nc.vector.tensor_tensor(
            out=t1[:], in0=x2t[:, :, 0:64], in1=x2t[:, :, 64:128], op=ALU.add
        )
        nc.vector.tensor_tensor(
            out=t2[:], in0=t1[:, :, 0:32], in1=t1[:, :, 32:64], op=ALU.add
        )

    def st_tree34(n):
        st = state[n]
        t2 = st["t2"]
        t3 = scrpool.tile([128, NCH, 16], bf16, tag="t3", bufs=2)
        t4 = scrpool.tile([128, NCH, 8], bf16, tag="t4", bufs=2)
        ss = smalls.tile([128, NCH], f32, tag="ss")
        st["ss"] = ss
        nc.vector.tensor_tensor(
            out=t3[:], in0=t2[:, :, 0:16], in1=t2[:, :, 16:32], op=ALU.add
        )
        nc.vector.tensor_tensor(
            out=t4[:], in0=t3[:, :, 0:8], in1=t3[:, :, 8:16], op=ALU.add
        )
        nc.vector.tensor_reduce(out=ss[:], in_=t4[:], axis=X_AX, op=ALU.add)

    def st_ischain(n):
        st = state[n]
        lns = smalls.tile([128, NCH], f32, tag="lns")
        isb = smalls.tile([128, NCH], bf16, tag="isb")
        st["isb"] = isb
        nc.scalar.activation(lns[:], st["ss"][:], AF.Ln)
        nc.scalar.activation(isb[:], lns[:], AF.Exp, scale=-0.5)

    def st_scol(n):
        st = state[n]
        # s = ss * inv_s = ||x_p||, into col 128 of each xtr chunk
        nc.gpsimd.tensor_tensor(
            out=st["xtr"][:, :, 128], in0=st["ss"][:], in1=st["isb"][:], op=ALU.mult
        )

    # ---- stage C: scaled logits (sample lag 2, Pool + DVE halves) ----
    def st_slg(n, half, eng):
        st = state[n]
        if "slgt" not in st:
            st["slgt"] = scrpool.tile(
                [128, NCH, K], bf16, tag="slgt", name="slgt", bufs=2
            )
        c0, c1 = (0, NCH // 2) if half == 0 else (NCH // 2, NCH)
        eng.tensor_tensor(
            out=st["slgt"][:, c0:c1, :],
            in0=st["lgs"][:, c0:c1, :],
            in1=bcast(st["isb"][:, c0:c1], K),
            op=ALU.mult,
        )

    # ---- stage D: exp (sample lag 3, ACT) ----
    def st_exp(n, half):
        st = state[n]
        if "et" not in st:
            st["et"] = ebpool.tile([128, NCH, K], bf16, tag="et", name="et")
        c0, c1 = (0, NCH // 2) if half == 0 else (NCH // 2, NCH)
        nc.scalar.activation(
            st["et"][:, c0:c1, :], st["slgt"][:, c0:c1, :], AF.Exp
        )

    # ---- stage E: Z, 1/Z, t = inv_s/Z, sb = e*t (sample lag 4) ----
    def st_zchain(n):
        st = state[n]
        et = st["et"]
        z1 = scrpool.tile([128, NCH, 32], bf16, tag="z1", bufs=2)
        z2 = scrpool.tile([128, NCH, 16], bf16, tag="z2", bufs=2)
        z3 = scrpool.tile([128, NCH, 8], bf16, tag="z3", bufs=2)
        zz = smalls.tile([128, NCH], f32, tag="zz")
        st["zz"] = zz
        nc.vector.tensor_tensor(
            out=z1[:], in0=et[:, :, 0:32], in1=et[:, :, 32:64], op=ALU.add
        )
        nc.vector.tensor_tensor(
            out=z2[:], in0=z1[:, :, 0:16], in1=z1[:, :, 16:32], op=ALU.add
        )
        nc.vector.tensor_tensor(
            out=z3[:], in0=z2[:, :, 0:8], in1=z2[:, :, 8:16], op=ALU.add
        )
        nc.vector.tensor_reduce(out=zz[:], in_=z3[:], axis=X_AX, op=ALU.add)

    def st_zchain_h(n, half):
        st = state[n]
        et = st["et"]
        if half == 0:
            st["z1"] = scrpool.tile([128, NCH, 32], bf16, tag="z1", bufs=2, name="z1")
            st["z2"] = scrpool.tile([128, NCH, 16], bf16, tag="z2", bufs=2, name="z2")
            st["z3"] = scrpool.tile([128, NCH, 8], bf16, tag="z3", bufs=2, name="z3")
            st["zz"] = smalls.tile([128, NCH], f32, tag="zz", name="zz")
        c0, c1 = (0, NCH // 2) if half == 0 else (NCH // 2, NCH)
        z1, z2, z3 = st["z1"], st["z2"], st["z3"]
        nc.vector.tensor_tensor(
            out=z1[:, c0:c1, :],
            in0=et[:, c0:c1, 0:32],
            in1=et[:, c0:c1, 32:64],
            op=ALU.add,
        )
        nc.vector.tensor_tensor(
            out=z2[:, c0:c1, :],
            in0=z1[:, c0:c1, 0:16],
            in1=z1[:, c0:c1, 16:32],
            op=ALU.add,
        )
        nc.vector.tensor_tensor(
            out=z3[:, c0:c1, :],
            in0=z2[:, c0:c1, 0:8],
            in1=z2[:, c0:c1, 8:16],
            op=ALU.add,
        )
        nc.vector.tensor_reduce(
            out=st["zz"][:, c0:c1], in_=z3[:, c0:c1, :], axis=X_AX, op=ALU.add
        )

    def st_recip_h(n, half):
        st = state[n]
        if half == 0:
            st["rr"] = smalls.tile([128, NCH], f32, tag="rr", name="rr")
        c0, c1 = (0, NCH // 2) if half == 0 else (NCH // 2, NCH)
        nc.vector.reciprocal(st["rr"][:, c0:c1], st["zz"][:, c0:c1])

    def st_tsc_h(n, half):
        st = state[n]
        if half == 0:
            st["tsc"] = smalls.tile([128, NCH], bf16, tag="tsc", name="tsc")
        c0, c1 = (0, NCH // 2) if half == 0 else (NCH // 2, NCH)
        nc.vector.tensor_tensor(
            out=st["tsc"][:, c0:c1],
            in0=st["isb"][:, c0:c1],
            in1=st["rr"][:, c0:c1],
            op=ALU.mult,
        )

    def st_recip(n):
        st = state[n]
        rr = smalls.tile([128, NCH], f32, tag="rr")
        st["rr"] = rr
        nc.vector.reciprocal(rr[:], st["zz"][:])

    def st_tsc(n):
        st = state[n]
        tsc = smalls.tile([128, NCH], bf16, tag="tsc")
        st["tsc"] = tsc
        nc.gpsimd.tensor_tensor(
            out=tsc[:], in0=st["isb"][:], in1=st["rr"][:], op=ALU.mult
        )

    def st_sbt(n, half, eng):
        st = state[n]
        if "sbt" not in st:
            st["sbt"] = sbtpool.tile([128, NCH, K], bf16, tag="sbt", name="sbt")
        c0, c1 = (0, NCH // 2) if half == 0 else (NCH // 2, NCH)
        eng.tensor_tensor(
            out=st["sbt"][:, c0:c1, :],
            in0=st["et"][:, c0:c1, :],
            in1=bcast(st["tsc"][:, c0:c1], K),
            op=ALU.mult,
        )

    cstate = {}  # open accumulation tiles for interleaved pass C

    def emit_passC_chunks(n, c0, c1):
        st = state[n]
        xtr, sbt = st["xtr"], st["sbt"]
        if n not in cstate:
            cstate[n] = pp_acc.tile([K, 129], f32, tag="acc", name="acc")
        acc_p = cstate[n]
        for c in range(c0, min(c1, NCH)):
            p0, w = CHUNKS[c]
            nc.tensor.matmul(
                acc_p[:, :],
                lhsT=sbt[:w, c, :],
                rhs=xtr[:w, c, :],
                start=(c == 0),
                stop=(c == NCH - 1),
            )

    def finish_passC(n, agg_all, ssa_all):
        acc_p = cstate.pop(n)
        state.pop(n)
        # evacuate: agg = cols 0:128; sum_sa = col 128
        nc.vector.tensor_copy(agg_all[:, n, :], acc_p[:, 0:D])
        nc.scalar.copy(ssa_all[:, n : n + 1], acc_p[:, 128:129])

    # batched across all samples
    agg_all = tails.tile([K, NS, D], f32)
    ssa_all = tails.tile([K, NS], f32)

    def emit_tail(n0, n1):
        nn = n1 - n0
        agg_h = agg_all[:, n0:n1, :]
        ssa_h = ssa_all[:, n0:n1]
        vl = tails.tile([K, nn, D], f32, tag="t_vl", bufs=2)
        vsq = tails.tile([K, nn * D], f32, tag="t_vsq", bufs=2)
        q = tails.tile([K, nn], f32, tag="t_q", bufs=2)
        qm = tails.tile([K, nn], f32, tag="t_qm", bufs=2)
        isq = tails.tile([K, nn], f32, tag="t_isq", bufs=2)
        isq2 = tails.tile([K, nn], f32, tag="t_isq2", bufs=2)
        u = tails.tile([K, nn], f32, tag="t_u", bufs=2)
        gisr = tails.tile([1, nn], f32, tag="t_gisr", bufs=2)
        gb = tails.tile([K, nn], f32, tag="t_gb", bufs=2)
        sall = tails.tile([K, nn], f32, tag="t_s", bufs=2)
        vf = tails.tile([K, nn, D], f32, tag="t_vf", bufs=2)

        # vl = agg - ssa * cent
        nc.gpsimd.tensor_tensor(
            out=vl[:], in0=bcast(ssa_h, D), in1=mid_bcast(cent_s[:], nn), op=ALU.mult
        )
        nc.vector.tensor_tensor(out=vl[:], in0=agg_h, in1=vl[:], op=ALU.subtract)
        # q = rowsum(vl^2) per (k, n)
        vsqv = vsq[:].rearrange("k (n d) -> k n d", n=nn)
        nc.scalar.activation(vsqv, vl[:], AF.Square)
        nc.vector.tensor_reduce(out=q[:], in_=vsqv, axis=X_AX, op=ALU.add)
        nc.vector.tensor_scalar_max(qm[:], q[:], 1e-24)
        lq = tails.tile([K, nn], f32, tag="t_lq", bufs=2)
        nc.scalar.activation(lq[:], qm[:], AF.Ln)
        nc.scalar.activation(isq[:], lq[:], AF.Exp, scale=-0.5)
        # g = sum_k q_k * isq_k^2  (per sample)
        nc.vector.tensor_tensor(out=isq2[:], in0=isq[:], in1=isq[:], op=ALU.mult)
        nc.vector.tensor_tensor(out=u[:], in0=q[:], in1=isq2[:], op=ALU.mult)
        g_p = pp_tiny.tile([NS, 1], f32, tag="tiny")
        nc.tensor.matmul(
            g_p[:nn, :], lhsT=u[:], rhs=ones_col[:], start=True, stop=True
        )
        # gis = g^-0.5 -> transpose to a row -> broadcast over k partitions
        gm = tails.tile([nn, 1], f32, tag="t_gm", bufs=2)
        nc.vector.tensor_scalar_max(gm[:], g_p[:nn, :], 1e-24)
        gis = tails.tile([nn, 1], f32, tag="t_gis", bufs=2)
        lgm = tails.tile([nn, 1], f32, tag="t_lgm", bufs=2)
        nc.scalar.activation(lgm[:], gm[:], AF.Ln)
        nc.scalar.activation(gis[:], lgm[:], AF.Exp, scale=-0.5)
        gr_p = pp_tiny.tile([1, NS], f32, tag="tiny")
        nc.tensor.matmul(
            gr_p[:, :nn], lhsT=gis[:], rhs=ident[:nn, :nn], start=True, stop=True
        )
        nc.vector.tensor_copy(gisr[:], gr_p[:, :nn])
        gb_p = pp_tiny.tile([K, NS], f32, tag="tiny")
        nc.tensor.matmul(
            gb_p[:, :nn], lhsT=ones_row[:], rhs=gisr[:], start=True, stop=True
        )
        nc.vector.tensor_copy(gb[:], gb_p[:, :nn])
        # s = isq * gb; vf = vl * s
        nc.vector.tensor_tensor(out=sall[:], in0=isq[:], in1=gb[:], op=ALU.mult)
        nc.gpsimd.tensor_tensor(out=vf[:], in0=vl[:], in1=bcast(sall[:], D), op=ALU.mult)
        nc.sync.dma_start(
            out=out_ap.rearrange("n k d -> k n d")[:, n0:n1, :], in_=vf[:]
        )

    # 5-stage software pipeline, one stage lag per round; every stage
    # consumes only tensors finished a full round earlier (no intra-round
    # cross-engine waits):
    #   round r: DMA prefetch(r+1) | passA+evacs(r) | B: ss/inv_s(r-1) |
    #   C: slg(r-2, Pool) | D: exp(r-3, ACT) | E: Z/recip/sb(r-4, DVE) |
    #   passC(r-5) interleaved on PE.
    emit_dma(0)
    emit_dma(1)
    pending_fin = None
    for r in range(NS + 2):
        na = r if r < NS else None
        if na is not None and r + 2 < NS:
            emit_dma(r + 2)
        if na is not None:
            alloc_sample(na)
        nb = r - 1 if 0 <= r - 1 < NS else None
        sd = r - 2 if 0 <= r - 2 < NS else None
        pe = r - 3 if 0 <= r - 3 < NS else None
        # finish the previous round's accumulation at the HEAD of this
        # round's queues so the acc bank frees before passC(pe) needs it
        if pending_fin is not None:
            finish_passC(pending_fin, agg_all, ssa_all)
            if pending_fin % 2 == 1:
                emit_tail(pending_fin - 1, pending_fin + 1)
            pending_fin = None
        if na is None:
            # drain rounds: no passA to hide behind -- run the remaining
            # chains latency-ordered at half-sample granularity so the
            # engines pipeline against each other; tails go dead last
            if pe is not None:
                emit_passC_chunks(pe, 0, NCH)
                finish_passC(pe, agg_all, ssa_all)
            if sd is not None:
                st_exp(sd, 0)
                st_exp(sd, 1)
            if nb is not None:
                # feeds next round's exp: emit before the sd chain on DVE
                st_sq(nb)
                st_tree12(nb)
                st_tree34(nb)
                st_ischain(nb)
                st_slg(nb, 0, nc.vector)
                st_slg(nb, 1, nc.vector)
                st_scol(nb)
            if sd is not None:
                st_zchain_h(sd, 0)
                st_recip_h(sd, 0)
                st_tsc_h(sd, 0)
                st_sbt(sd, 0, nc.vector)
                st_zchain_h(sd, 1)
                st_recip_h(sd, 1)
                st_tsc_h(sd, 1)
                st_sbt(sd, 1, nc.vector)
            if sd == NS - 1:
                # last sample: its sb is ready now; aggregate immediately
                emit_passC_chunks(sd, 0, NCH)
                finish_passC(sd, agg_all, ssa_all)
            if pe is not None and pe % 2 == 1:
                emit_tail(pe - 1, pe + 1)
            if sd == NS - 1:
                emit_tail(NS - 2, NS)
            continue
        for g in range(NG):
            # passC first: it is always ready (sbt a round old), so the PE
            # drains it while a late xs DMA finishes instead of idling
            if pe is not None:
                emit_passC_chunks(pe, *pc_slices[g])
            if na is not None:
                xt_p, gc, gn = emit_passA_group(na, g)
                emit_evacs(na, g, xt_p, gc, gn)
            if g == 0 and nb is not None:
                st_sq(nb)
            elif g == 1:
                if sd is not None:
                    st_exp(sd, 0)
                if nb is not None:
                    st_tree12(nb)
            elif g == 2:
                if sd is not None:
                    st_exp(sd, 1)
                if nb is not None:
                    st_tree34(nb)
            elif g == 3:
                if sd is not None:
                    st_zchain(sd)
                    st_recip(sd)
                if nb is not None:
                    st_ischain(nb)
            elif g == 4:
                if sd is not None:
                    st_tsc(sd)
                if nb is not None:
                    # last two samples sit on the drain critical path: Pool's
                    # slow bcast mult would gate exp -> Z -> sb -> passC
                    fast = na is None or nb >= NS - 2
                    st_slg(nb, 0, nc.vector if fast else nc.gpsimd)
                    st_slg(nb, 1, nc.vector)
            elif g == 5:
                if sd is not None:
                    fastb = na is None or sd >= NS - 2
                    st_sbt(sd, 0, nc.vector if fastb else nc.gpsimd)
                    st_sbt(sd, 1, nc.vector)
                if nb is not None:
                    st_scol(nb)
        if pe is not None:
            pending_fin = pe


def kernel(x, conv_w, centroids):
    from concourse.bass_utils import run_bass_kernel_spmd

    if "nc" not in _CACHE:
        _CACHE["nc"] = _build_nc()
    nc = _CACHE["nc"]

    x = np.ascontiguousarray(np.asarray(x, dtype=np.float32).reshape(N, D, HW))
    wt = np.ascontiguousarray(np.asarray(conv_w, dtype=np.float32).T)
    cent = np.ascontiguousarray(np.asarray(centroids, dtype=np.float32))
    idw = np.zeros((D, 256), dtype=np.float32)
    idw[:, 0:128] = np.eye(D, dtype=np.float32)
    idw[:, 128:192] = wt
    in_maps = [
        {"x": x[i * NS : (i + 1) * NS], "wt": wt, "cent": cent, "idw": idw}
        for i in range(NCORES)
    ]
    res = run_bass_kernel_spmd(nc, in_maps, core_ids=list(range(NCORES))).results
    out = np.concatenate([r["out"].reshape(NS, K * D) for r in res], axis=0)
    return out


if __name__ == "__main__":
    rng = np.random.default_rng(0)
    xs = rng.standard_normal((N, D, 60, 80), dtype=np.float32)
    cw = (rng.standard_normal((K, D)) * 0.1).astype(np.float32)
    ct = rng.random((K, D), dtype=np.float32)
    o = kernel(x=xs, conv_w=cw, centroids=ct)
    print("kernel out", o.shape, o.dtype, np.abs(o).max())



# revision 39
# speedup vs baseline: 1.0118x; 1.0118x over previous
"""NetVLAD Trainium2 Bass kernel.

Full inputs in, full output out. Data-parallel over batch N=64 across 8
NeuronCores (8 samples per core); conv weight and centroids replicated.

Per-sample algorithm (mathematically equal to the reference, never
materializing the channel-normalized x):
  X = x[n]  [D=128, P=4800]  (D on SBUF partitions, contiguous in HBM)
  For each 128-wide chunk of P (p on partitions after a PE transpose):
    ss[p]    = sum_d X[d,p]^2
    inv_s    = ss^-0.5                   (DVE pow — keeps the ACT table
                                          set fixed: only Copy/Square/Exp)
    logitsT  = X_c^T @ Wt                (PE)
    e        = exp(logitsT * inv_s)      (softmax max-subtraction skipped:
                                          |logits*inv_s| <= ~1.2)
    sb       = e * (inv_s / Z),  Z = sum_k e
    acc     += [sb | e]^T @ [X_c^T | 1/Z]   (PE, PSUM accumulate)
  agg      = acc[0:64, 0:128];  sum_sa = acc[64:128, 128]
  vlad     = agg - sum_sa * centroids, then intra + global L2 norm.

Pipelining: per-chunk scalar work is batched into whole-sample ops
(one Square, one reduce, one Exp, ...); the accumulate matmuls of
sample n-2 are emitted between pass A of sample n so the PE never
waits on the scalar chain. The [sb|e] and [XT|1/Z] operands are bf16
(FWL fast weight load; f32 PSUM accumulation).
"""

import sys

if "/opt/trn_rl_repo" not in sys.path:
    sys.path.insert(0, "/opt/trn_rl_repo")

import numpy as np
from contextlib import ExitStack

N, D, HW, K = 64, 128, 4800, 64
NCORES = 8
NS = N // NCORES  # samples per core

CHUNKS = [(i * 128, min(128, HW - i * 128)) for i in range((HW + 127) // 128)]
NCH = len(CHUNKS)  # 38: 37 full + one 64-wide

_CACHE = {}


def _patch_act_tables():
    """Steer bacc's ACT table-set placement to the one set that covers
    every function we use (ln/exp/square/copy) so the kernel pays a single
    ACT_TABLE_LOAD instead of thrashing between per-anchor sets."""
    if _CACHE.get("act_patched"):
        return
    from concourse import bacc, mybir

    orig = bacc.get_activation_tables
    AF = mybir.ActivationFunctionType
    combo = "natural_log_exp_and_others"

    def patched(arch):
        t = {k: set(v) for k, v in orig(arch).items()}
        if combo in t:
            for name in t:
                if name != combo:
                    t[name] = t[name] - {AF.Ln, AF.Exp}
        return t

    bacc.get_activation_tables = patched
    _CACHE["act_patched"] = True


def _build_nc():
    import concourse.tile as tile
    from concourse import bacc, mybir

    _patch_act_tables()

    nc = bacc.Bacc(
        "TRN2",
        target_bir_lowering=False,
        debug=False,
        enable_asserts=False,
        num_devices=NCORES,
    )
    x_ap = nc.dram_tensor(
        "x", [NS, D, HW], mybir.dt.float32r, kind="ExternalInput"
    ).ap()
    wt_ap = nc.dram_tensor("wt", [D, K], mybir.dt.float32, kind="ExternalInput").ap()
    idw_ap = nc.dram_tensor(
        "idw", [D, 256], mybir.dt.float32r, kind="ExternalInput"
    ).ap()
    cent_ap = nc.dram_tensor(
        "cent", [K, D], mybir.dt.float32, kind="ExternalInput"
    ).ap()
    out_ap = nc.dram_tensor(
        "out", [NS, K, D], mybir.dt.float32, kind="ExternalOutput"
    ).ap()

    with tile.TileContext(nc) as tc:
        with ExitStack() as ctx:
            _body(ctx, tc, out_ap, x_ap, wt_ap, cent_ap, idw_ap)
    nc.compile()
    return nc


def _body(ctx, tc, out_ap, x_ap, wt_ap, cent_ap, idw_ap):
    import concourse.bass as bass
    from concourse import masks, mybir

    nc = tc.nc
    f32 = mybir.dt.float32
    bf16 = mybir.dt.bfloat16
    AF = mybir.ActivationFunctionType
    ALU = mybir.AluOpType
    X_AX = mybir.AxisListType.X

    singles = ctx.enter_context(tc.tile_pool(name="singles", bufs=1))
    xpool = ctx.enter_context(tc.tile_pool(name="xpool", bufs=3))
    xtrpool = ctx.enter_context(tc.tile_pool(name="xtrpool", bufs=5))
    ebpool = ctx.enter_context(tc.tile_pool(name="ebpool", bufs=2))
    sbtpool = ctx.enter_context(tc.tile_pool(name="sbtpool", bufs=2))
    lpool = ctx.enter_context(tc.tile_pool(name="lpool", bufs=3))
    scrpool = ctx.enter_context(tc.tile_pool(name="scrpool", bufs=2))
    smalls = ctx.enter_context(tc.tile_pool(name="smalls", bufs=6))
    tails = ctx.enter_context(tc.tile_pool(name="tails", bufs=1))
    pp_xt = ctx.enter_context(tc.tile_pool(name="pp_xt", bufs=2, space="PSUM"))
    pp_acc = ctx.enter_context(tc.tile_pool(name="pp_acc", bufs=1, space="PSUM"))
    pp_tiny = ctx.enter_context(tc.tile_pool(name="pp_tiny", bufs=1, space="PSUM"))

    def bcast(ap, n):
        # append a step-0 free dim: [..., n] broadcast view
        return bass.AP(tensor=ap.tensor, offset=ap.offset, ap=list(ap.ap) + [[0, n]])

    def mid_bcast(ap, n):
        # [p, f] -> [p, n, f] with step-0 middle dim
        return bass.AP(
            tensor=ap.tensor,
            offset=ap.offset,
            ap=[ap.ap[0], [0, n]] + list(ap.ap[1:]),
        )

    # constants
    f32r = mybir.dt.float32r
    ident = singles.tile([128, 128], f32)
    masks.make_identity(nc, ident[:])
    # fused rhs for pass A: [identity | Wt | zero-pad] — one matmul yields
    # [X_c^T | logits | junk]. Padded to 256 cols so the fp32r (single-pass
    # fp32) matmul streams at 1 cycle/row instead of fp32's 4. The tile is
    # declared float32r so every producer writes fp32r (BIR verifier rule).
    identwt = singles.tile([128, 256], f32r)
    nc.sync.dma_start(out=identwt[:], in_=idw_ap[:])
    cent_s = singles.tile([K, D], f32)
    nc.sync.dma_start(out=cent_s[:], in_=cent_ap[:])
    ones_col = singles.tile([K, 1], f32)
    nc.vector.memset(ones_col[:], 1.0)
    ones_row = singles.tile([1, K], f32)
    nc.vector.memset(ones_row[:], 1.0)

    GRP = 6  # fused-matmul chunks per PSUM tile (3 banks; 2 rotating bufs)
    groups = []
    c0 = 0
    while c0 < NCH:
        groups.append(list(range(c0, min(c0 + GRP, NCH))))
        c0 += GRP
    NG = len(groups)  # 7

    # pass-C interleave: chunk range emitted after group g of the round
    pc_slices = []
    base = 0
    for g in range(NG):
        take = (NCH - base + (NG - g) - 1) // (NG - g)
        pc_slices.append((base, base + take))
        base += take

    state = {}  # per-sample live tiles

    def emit_dma(n):
        xs = xpool.tile([D, HW], mybir.dt.float32r, tag="xs")
        if n == 0:
            # per-group pieces: group-0 matmuls start after ~1/7 of the load
            for grp in groups:
                p0 = CHUNKS[grp[0]][0]
                p1 = CHUNKS[grp[-1]][0] + CHUNKS[grp[-1]][1]
                nc.sync.dma_start(out=xs[:, p0:p1], in_=x_ap[n, :, p0:p1])
        else:
            nc.sync.dma_start(out=xs[:, 0 : HW // 2], in_=x_ap[n, :, 0 : HW // 2])
            nc.sync.dma_start(out=xs[:, HW // 2 :], in_=x_ap[n, :, HW // 2 :])
        state.setdefault(n, {})["xs"] = xs

    def alloc_sample(n):
        st = state.setdefault(n, {})
        # [XT | s] per chunk (bf16): cols 0:128 = X_c^T, col 128 = ||x_p||
        st["xtr"] = xtrpool.tile([128, NCH, 129], bf16, tag="xtr", name="xtr")
        # raw logits [p, c, k]
        st["lgs"] = lpool.tile([128, NCH, K], bf16, tag="lgs", name="lgs")

    def emit_passA_group(n, g):
        st = state[n]
        xs = st["xs"]
        grp = groups[g]
        xt_p = pp_xt.tile([128, GRP, 256], f32, tag="xt")
        for j, c in enumerate(grp):
            p0, w = CHUNKS[c]
            nc.tensor.matmul(
                xt_p[:w, j, 0:256],
                lhsT=xs[:, p0 : p0 + w],
                rhs=identwt[:],
                start=True,
                stop=True,
            )
        return xt_p, grp[0], len(grp)

    def emit_evacs(n, g, xt_p, gc, gn):
        st = state[n]
        xtr, lgs = st["xtr"], st["lgs"]
        # X^T columns -> SBUF bf16 (one consolidated ACT copy per tile)
        nc.scalar.copy(xtr[:, gc : gc + gn, 0:128], xt_p[:, 0:gn, 0:128])
        # logits -> lgs [p, c, k]; two tiles' worth on DVE, rest on ACT
        if g in (1, 3):
            nc.vector.tensor_copy(lgs[:, gc : gc + gn, :], xt_p[:, 0:gn, 128:192])
        else:
            nc.scalar.copy(lgs[:, gc : gc + gn, :], xt_p[:, 0:gn, 128:192])

    # ---- stage B: ss = rowsum(X_c^T^2), inv_s, s-col (sample lag 1) ----
    def st_sq(n):
        st = state[n]
        x2t = scrpool.tile([128, NCH, 128], bf16, tag="x2t", bufs=1)
        st["x2t"] = x2t
        xv = st["xtr"][:, :, 0:128]
        nc.vector.tensor_tensor(out=x2t[:], in0=xv, in1=xv, op=ALU.mult)

    def st_tree12(n):
        st = state[n]
        x2t = st["x2t"]
        t1 = scrpool.tile([128, NCH, 64], bf16, tag="t1", bufs=2)
        t2 = scrpool.tile([128, NCH, 32], bf16, tag="t2", bufs=2)
        st["t2"] = t2
        nc.vector.tensor_tensor(
            out=t1[:], in0=x2t[:, :, 0:64], in1=x2t[:, :, 64:128], op=ALU.add
        )
        nc.vector.tensor_tensor(
            out=t2[:], in0=t1[:, :, 0:32], in1=t1[:, :, 32:64], op=ALU.add
        )

    def st_tree34(n):
        st = state[n]
        t2 = st["t2"]
        t3 = scrpool.tile([128, NCH, 16], bf16, tag="t3", bufs=2)
        t4 = scrpool.tile([128, NCH, 8], bf16, tag="t4", bufs=2)
        ss = smalls.tile([128, NCH], f32, tag="ss")
        st["ss"] = ss
        nc.vector.tensor_tensor(
            out=t3[:], in0=t2[:, :, 0:16], in1=t2[:, :, 16:32], op=ALU.add
        )
        nc.vector.tensor_tensor(
            out=t4[:], in0=t3[:, :, 0:8], in1=t3[:, :, 8:16], op=ALU.add
        )
        nc.vector.tensor_reduce(out=ss[:], in_=t4[:], axis=X_AX, op=ALU.add)

    def st_ischain(n):
        st = state[n]
        lns = smalls.tile([128, NCH], f32, tag="lns")
        isb = smalls.tile([128, NCH], bf16, tag="isb")
        st["isb"] = isb
        nc.scalar.activation(lns[:], st["ss"][:], AF.Ln)
        nc.scalar.activation(isb[:], lns[:], AF.Exp, scale=-0.5)

    def st_scol(n):
        st = state[n]
        # s = ss * inv_s = ||x_p||, into col 128 of each xtr chunk
        nc.gpsimd.tensor_tensor(
            out=st["xtr"][:, :, 128], in0=st["ss"][:], in1=st["isb"][:], op=ALU.mult
        )

    # ---- stage C: scaled logits (sample lag 2, Pool + DVE halves) ----
    def st_slg(n, half, eng):
        st = state[n]
        if "slgt" not in st:
            st["slgt"] = scrpool.tile(
                [128, NCH, K], bf16, tag="slgt", name="slgt", bufs=2
            )
        c0, c1 = (0, NCH // 2) if half == 0 else (NCH // 2, NCH)
        eng.tensor_tensor(
            out=st["slgt"][:, c0:c1, :],
            in0=st["lgs"][:, c0:c1, :],
            in1=bcast(st["isb"][:, c0:c1], K),
            op=ALU.mult,
        )

    # ---- stage D: exp (sample lag 3, ACT) ----
    def st_exp(n, half):
        st = state[n]
        if "et" not in st:
            st["et"] = ebpool.tile([128, NCH, K], bf16, tag="et", name="et")
        c0, c1 = (0, NCH // 2) if half == 0 else (NCH // 2, NCH)
        nc.scalar.activation(
            st["et"][:, c0:c1, :], st["slgt"][:, c0:c1, :], AF.Exp
        )

    # ---- stage E: Z, 1/Z, t = inv_s/Z, sb = e*t (sample lag 4) ----
    def st_zchain(n):
        st = state[n]
        et = st["et"]
        z1 = scrpool.tile([128, NCH, 32], bf16, tag="z1", bufs=2)
        z2 = scrpool.tile([128, NCH, 16], bf16, tag="z2", bufs=2)
        z3 = scrpool.tile([128, NCH, 8], bf16, tag="z3", bufs=2)
        zz = smalls.tile([128, NCH], f32, tag="zz")
        st["zz"] = zz
        nc.vector.tensor_tensor(
            out=z1[:], in0=et[:, :, 0:32], in1=et[:, :, 32:64], op=ALU.add
        )
        nc.vector.tensor_tensor(
            out=z2[:], in0=z1[:, :, 0:16], in1=z1[:, :, 16:32], op=ALU.add
        )
        nc.vector.tensor_tensor(
            out=z3[:], in0=z2[:, :, 0:8], in1=z2[:, :, 8:16], op=ALU.add
        )
        nc.vector.tensor_reduce(out=zz[:], in_=z3[:], axis=X_AX, op=ALU.add)

    def st_recip(n):
        st = state[n]
        rr = smalls.tile([128, NCH], f32, tag="rr")
        st["rr"] = rr
        nc.vector.reciprocal(rr[:], st["zz"][:])

    def st_tsc(n):
        st = state[n]
        tsc = smalls.tile([128, NCH], bf16, tag="tsc")
        st["tsc"] = tsc
        nc.gpsimd.tensor_tensor(
            out=tsc[:], in0=st["isb"][:], in1=st["rr"][:], op=ALU.mult
        )

    def st_sbt(n, half, eng):
        st = state[n]
        if "sbt" not in st:
            st["sbt"] = sbtpool.tile([128, NCH, K], bf16, tag="sbt", name="sbt")
        c0, c1 = (0, NCH // 2) if half == 0 else (NCH // 2, NCH)
        eng.tensor_tensor(
            out=st["sbt"][:, c0:c1, :],
            in0=st["et"][:, c0:c1, :],
            in1=bcast(st["tsc"][:, c0:c1], K),
            op=ALU.mult,
        )

    cstate = {}  # open accumulation tiles for interleaved pass C

    def emit_passC_chunks(n, c0, c1):
        st = state[n]
        xtr, sbt = st["xtr"], st["sbt"]
        if n not in cstate:
            cstate[n] = pp_acc.tile([K, 129], f32, tag="acc", name="acc")
        acc_p = cstate[n]
        for c in range(c0, min(c1, NCH)):
            p0, w = CHUNKS[c]
            nc.tensor.matmul(
                acc_p[:, :],
                lhsT=sbt[:w, c, :],
                rhs=xtr[:w, c, :],
                start=(c == 0),
                stop=(c == NCH - 1),
            )

    def finish_passC(n, agg_all, ssa_all):
        acc_p = cstate.pop(n)
        state.pop(n)
        # evacuate: agg = cols 0:128; sum_sa = col 128
        nc.vector.tensor_copy(agg_all[:, n, :], acc_p[:, 0:D])
        nc.scalar.copy(ssa_all[:, n : n + 1], acc_p[:, 128:129])

    # batched across all samples
    agg_all = tails.tile([K, NS, D], f32)
    ssa_all = tails.tile([K, NS], f32)

    def emit_tail(n0, n1):
        nn = n1 - n0
        agg_h = agg_all[:, n0:n1, :]
        ssa_h = ssa_all[:, n0:n1]
        vl = tails.tile([K, nn, D], f32, tag="t_vl", bufs=2)
        vsq = tails.tile([K, nn * D], f32, tag="t_vsq", bufs=2)
        q = tails.tile([K, nn], f32, tag="t_q", bufs=2)
        qm = tails.tile([K, nn], f32, tag="t_qm", bufs=2)
        isq = tails.tile([K, nn], f32, tag="t_isq", bufs=2)
        isq2 = tails.tile([K, nn], f32, tag="t_isq2", bufs=2)
        u = tails.tile([K, nn], f32, tag="t_u", bufs=2)
        gisr = tails.tile([1, nn], f32, tag="t_gisr", bufs=2)
        gb = tails.tile([K, nn], f32, tag="t_gb", bufs=2)
        sall = tails.tile([K, nn], f32, tag="t_s", bufs=2)
        vf = tails.tile([K, nn, D], f32, tag="t_vf", bufs=2)

        # vl = agg - ssa * cent
        nc.gpsimd.tensor_tensor(
            out=vl[:], in0=bcast(ssa_h, D), in1=mid_bcast(cent_s[:], nn), op=ALU.mult
        )
        nc.vector.tensor_tensor(out=vl[:], in0=agg_h, in1=vl[:], op=ALU.subtract)
        # q = rowsum(vl^2) per (k, n)
        vsqv = vsq[:].rearrange("k (n d) -> k n d", n=nn)
        nc.scalar.activation(vsqv, vl[:], AF.Square)
        nc.vector.tensor_reduce(out=q[:], in_=vsqv, axis=X_AX, op=ALU.add)
        nc.vector.tensor_scalar_max(qm[:], q[:], 1e-24)
        lq = tails.tile([K, nn], f32, tag="t_lq", bufs=2)
        nc.scalar.activation(lq[:], qm[:], AF.Ln)
        nc.scalar.activation(isq[:], lq[:], AF.Exp, scale=-0.5)
        # g = sum_k q_k * isq_k^2  (per sample)
        nc.vector.tensor_tensor(out=isq2[:], in0=isq[:], in1=isq[:], op=ALU.mult)
        nc.vector.tensor_tensor(out=u[:], in0=q[:], in1=isq2[:], op=ALU.mult)
        g_p = pp_tiny.tile([NS, 1], f32, tag="tiny")
        nc.tensor.matmul(
            g_p[:nn, :], lhsT=u[:], rhs=ones_col[:], start=True, stop=True
        )
        # gis = g^-0.5 -> transpose to a row -> broadcast over k partitions
        gm = tails.tile([nn, 1], f32, tag="t_gm", bufs=2)
        nc.vector.tensor_scalar_max(gm[:], g_p[:nn, :], 1e-24)
        gis = tails.tile([nn, 1], f32, tag="t_gis", bufs=2)
        lgm = tails.tile([nn, 1], f32, tag="t_lgm", bufs=2)
        nc.scalar.activation(lgm[:], gm[:], AF.Ln)
        nc.scalar.activation(gis[:], lgm[:], AF.Exp, scale=-0.5)
        gr_p = pp_tiny.tile([1, NS], f32, tag="tiny")
        nc.tensor.matmul(
            gr_p[:, :nn], lhsT=gis[:], rhs=ident[:nn, :nn], start=True, stop=True
        )
        nc.vector.tensor_copy(gisr[:], gr_p[:, :nn])
        gb_p = pp_tiny.tile([K, NS], f32, tag="tiny")
        nc.tensor.matmul(
            gb_p[:, :nn], lhsT=ones_row[:], rhs=gisr[:], start=True, stop=True
        )
        nc.vector.tensor_copy(gb[:], gb_p[:, :nn])
        # s = isq * gb; vf = vl * s
        nc.vector.tensor_tensor(out=sall[:], in0=isq[:], in1=gb[:], op=ALU.mult)
        nc.gpsimd.tensor_tensor(out=vf[:], in0=vl[:], in1=bcast(sall[:], D), op=ALU.mult)
        nc.sync.dma_start(
            out=out_ap.rearrange("n k d -> k n d")[:, n0:n1, :], in_=vf[:]
        )

    # 5-stage software pipeline, one stage lag per round; every stage
    # consumes only tensors finished a full round earlier (no intra-round
    # cross-engine waits):
    #   round r: DMA prefetch(r+1) | passA+evacs(r) | B: ss/inv_s(r-1) |
    #   C: slg(r-2, Pool) | D: exp(r-3, ACT) | E: Z/recip/sb(r-4, DVE) |
    #   passC(r-5) interleaved on PE.
    emit_dma(0)
    emit_dma(1)
    pending_fin = None
    for r in range(NS + 3):
        na = r if r < NS else None
        if na is not None and r + 2 < NS:
            emit_dma(r + 2)
        if na is not None:
            alloc_sample(na)
        nb = r - 1 if 0 <= r - 1 < NS else None
        sd = r - 2 if 0 <= r - 2 < NS else None
        pe = r - 3 if 0 <= r - 3 < NS else None
        # finish the previous round's accumulation at the HEAD of this
        # round's queues so the acc bank frees before passC(pe) needs it
        if pending_fin is not None:
            finish_passC(pending_fin, agg_all, ssa_all)
            if pending_fin % 2 == 1:
                emit_tail(pending_fin - 1, pending_fin + 1)
            pending_fin = None
        for g in range(NG):
            # passC first: it is always ready (sbt a round old), so the PE
            # drains it while a late xs DMA finishes instead of idling
            if pe is not None:
                emit_passC_chunks(pe, *pc_slices[g])
            if na is not None:
                xt_p, gc, gn = emit_passA_group(na, g)
                emit_evacs(na, g, xt_p, gc, gn)
            if g == 0 and nb is not None:
                st_sq(nb)
            elif g == 1:
                if sd is not None:
                    st_exp(sd, 0)
                if nb is not None:
                    st_tree12(nb)
            elif g == 2:
                if sd is not None:
                    st_exp(sd, 1)
                if nb is not None:
                    st_tree34(nb)
            elif g == 3:
                if sd is not None:
                    st_zchain(sd)
                    st_recip(sd)
                if nb is not None:
                    st_ischain(nb)
            elif g == 4:
                if sd is not None:
                    st_tsc(sd)
                if nb is not None:
                    # last two samples sit on the drain critical path: Pool's
                    # slow bcast mult would gate exp -> Z -> sb -> passC
                    fast = na is None or nb >= NS - 2
                    st_slg(nb, 0, nc.vector if fast else nc.gpsimd)
                    st_slg(nb, 1, nc.vector)
            elif g == 5:
                if sd is not None:
                    fastb = na is None or sd >= NS - 2
                    st_sbt(sd, 0, nc.vector if fastb else nc.gpsimd)
                    st_sbt(sd, 1, nc.vector)
                if nb is not None:
                    st_scol(nb)
        if pe is not None:
            pending_fin = pe
    if pending_fin is not None:
        n_last = pending_fin
        finish_passC(n_last, agg_all, ssa_all)
        emit_tail(n_last - 1, n_last + 1)


def kernel(x, conv_w, centroids):
    from concourse.bass_utils import run_bass_kernel_spmd

    if "nc" not in _CACHE:
        _CACHE["nc"] = _build_nc()
    nc = _CACHE["nc"]

    x = np.ascontiguousarray(np.asarray(x, dtype=np.float32).reshape(N, D, HW))
    wt = np.ascontiguousarray(np.asarray(conv_w, dtype=np.float32).T)
    cent = np.ascontiguousarray(np.asarray(centroids, dtype=np.float32))
    idw = np.zeros((D, 256), dtype=np.float32)
    idw[:, 0:128] = np.eye(D, dtype=np.float32)
    idw[:, 128:192] = wt
    in_maps = [
        {"x": x[i * NS : (i + 1) * NS], "wt": wt, "cent": cent, "idw": idw}
        for i in range(NCORES)
    ]
    res = run_bass_kernel_spmd(nc, in_maps, core_ids=list(range(NCORES))).results
    out = np.concatenate([r["out"].reshape(NS, K * D) for r in res], axis=0)
    return out


if __name__ == "__main__":
    rng = np.random.default_rng(0)
    xs = rng.standard_normal((N, D, 60, 80), dtype=np.float32)
    cw = (rng.standard_normal((K, D)) * 0.1).astype(np.float32)
    ct = rng.random((K, D), dtype=np.float32)
    o = kernel(x=xs, conv_w=cw, centroids=ct)
    print("kernel out", o.shape, o.dtype, np.abs(o).max())



# revision 40
# speedup vs baseline: 1.0386x; 1.0265x over previous
"""NetVLAD Trainium2 Bass kernel.

Full inputs in, full output out. Data-parallel over batch N=64 across 8
NeuronCores (8 samples per core); conv weight and centroids replicated.

Per-sample algorithm (mathematically equal to the reference, never
materializing the channel-normalized x):
  X = x[n]  [D=128, P=4800]  (D on SBUF partitions, contiguous in HBM)
  For each 128-wide chunk of P (p on partitions after a PE transpose):
    ss[p]    = sum_d X[d,p]^2
    inv_s    = ss^-0.5                   (DVE pow — keeps the ACT table
                                          set fixed: only Copy/Square/Exp)
    logitsT  = X_c^T @ Wt                (PE)
    e        = exp(logitsT * inv_s)      (softmax max-subtraction skipped:
                                          |logits*inv_s| <= ~1.2)
    sb       = e * (inv_s / Z),  Z = sum_k e
    acc     += [sb | e]^T @ [X_c^T | 1/Z]   (PE, PSUM accumulate)
  agg      = acc[0:64, 0:128];  sum_sa = acc[64:128, 128]
  vlad     = agg - sum_sa * centroids, then intra + global L2 norm.

Pipelining: per-chunk scalar work is batched into whole-sample ops
(one Square, one reduce, one Exp, ...); the accumulate matmuls of
sample n-2 are emitted between pass A of sample n so the PE never
waits on the scalar chain. The [sb|e] and [XT|1/Z] operands are bf16
(FWL fast weight load; f32 PSUM accumulation).
"""

import sys

if "/opt/trn_rl_repo" not in sys.path:
    sys.path.insert(0, "/opt/trn_rl_repo")

import numpy as np
from contextlib import ExitStack

N, D, HW, K = 64, 128, 4800, 64
NCORES = 8
NS = N // NCORES  # samples per core

CHUNKS = [(i * 128, min(128, HW - i * 128)) for i in range((HW + 127) // 128)]
NCH = len(CHUNKS)  # 38: 37 full + one 64-wide

_CACHE = {}


def _patch_act_tables():
    """Steer bacc's ACT table-set placement to the one set that covers
    every function we use (ln/exp/square/copy) so the kernel pays a single
    ACT_TABLE_LOAD instead of thrashing between per-anchor sets."""
    if _CACHE.get("act_patched"):
        return
    from concourse import bacc, mybir

    orig = bacc.get_activation_tables
    AF = mybir.ActivationFunctionType
    combo = "natural_log_exp_and_others"

    def patched(arch):
        t = {k: set(v) for k, v in orig(arch).items()}
        if combo in t:
            for name in t:
                if name != combo:
                    t[name] = t[name] - {AF.Ln, AF.Exp}
        return t

    bacc.get_activation_tables = patched
    _CACHE["act_patched"] = True


def _build_nc():
    import concourse.tile as tile
    from concourse import bacc, mybir

    _patch_act_tables()

    nc = bacc.Bacc(
        "TRN2",
        target_bir_lowering=False,
        debug=False,
        enable_asserts=False,
        num_devices=NCORES,
    )
    x_ap = nc.dram_tensor(
        "x", [NS, D, HW], mybir.dt.float32r, kind="ExternalInput"
    ).ap()
    wt_ap = nc.dram_tensor("wt", [D, K], mybir.dt.float32, kind="ExternalInput").ap()
    idw_ap = nc.dram_tensor(
        "idw", [D, 256], mybir.dt.float32r, kind="ExternalInput"
    ).ap()
    cent_ap = nc.dram_tensor(
        "cent", [K, D], mybir.dt.float32, kind="ExternalInput"
    ).ap()
    out_ap = nc.dram_tensor(
        "out", [NS, K, D], mybir.dt.float32, kind="ExternalOutput"
    ).ap()

    with tile.TileContext(nc) as tc:
        with ExitStack() as ctx:
            _body(ctx, tc, out_ap, x_ap, wt_ap, cent_ap, idw_ap)
    nc.compile()
    return nc


def _body(ctx, tc, out_ap, x_ap, wt_ap, cent_ap, idw_ap):
    import concourse.bass as bass
    from concourse import masks, mybir

    nc = tc.nc
    f32 = mybir.dt.float32
    bf16 = mybir.dt.bfloat16
    AF = mybir.ActivationFunctionType
    ALU = mybir.AluOpType
    X_AX = mybir.AxisListType.X

    singles = ctx.enter_context(tc.tile_pool(name="singles", bufs=1))
    xpool = ctx.enter_context(tc.tile_pool(name="xpool", bufs=3))
    xtrpool = ctx.enter_context(tc.tile_pool(name="xtrpool", bufs=5))
    ebpool = ctx.enter_context(tc.tile_pool(name="ebpool", bufs=2))
    sbtpool = ctx.enter_context(tc.tile_pool(name="sbtpool", bufs=2))
    lpool = ctx.enter_context(tc.tile_pool(name="lpool", bufs=3))
    scrpool = ctx.enter_context(tc.tile_pool(name="scrpool", bufs=2))
    smalls = ctx.enter_context(tc.tile_pool(name="smalls", bufs=6))
    tails = ctx.enter_context(tc.tile_pool(name="tails", bufs=1))
    pp_xt = ctx.enter_context(tc.tile_pool(name="pp_xt", bufs=2, space="PSUM"))
    pp_acc = ctx.enter_context(tc.tile_pool(name="pp_acc", bufs=1, space="PSUM"))
    pp_tiny = ctx.enter_context(tc.tile_pool(name="pp_tiny", bufs=1, space="PSUM"))

    def bcast(ap, n):
        # append a step-0 free dim: [..., n] broadcast view
        return bass.AP(tensor=ap.tensor, offset=ap.offset, ap=list(ap.ap) + [[0, n]])

    def mid_bcast(ap, n):
        # [p, f] -> [p, n, f] with step-0 middle dim
        return bass.AP(
            tensor=ap.tensor,
            offset=ap.offset,
            ap=[ap.ap[0], [0, n]] + list(ap.ap[1:]),
        )

    # constants
    f32r = mybir.dt.float32r
    ident = singles.tile([128, 128], f32)
    masks.make_identity(nc, ident[:])
    # fused rhs for pass A: [identity | Wt | zero-pad] — one matmul yields
    # [X_c^T | logits | junk]. Padded to 256 cols so the fp32r (single-pass
    # fp32) matmul streams at 1 cycle/row instead of fp32's 4. The tile is
    # declared float32r so every producer writes fp32r (BIR verifier rule).
    identwt = singles.tile([128, 256], f32r)
    nc.sync.dma_start(out=identwt[:], in_=idw_ap[:])
    cent_s = singles.tile([K, D], f32)
    nc.sync.dma_start(out=cent_s[:], in_=cent_ap[:])
    ones_col = singles.tile([K, 1], f32)
    nc.vector.memset(ones_col[:], 1.0)
    ones_row = singles.tile([1, K], f32)
    nc.vector.memset(ones_row[:], 1.0)

    GRP = 6  # fused-matmul chunks per PSUM tile (3 banks; 2 rotating bufs)
    groups = []
    c0 = 0
    while c0 < NCH:
        groups.append(list(range(c0, min(c0 + GRP, NCH))))
        c0 += GRP
    NG = len(groups)  # 7

    # pass-C interleave: chunk range emitted after group g of the round
    pc_slices = []
    base = 0
    for g in range(NG):
        take = (NCH - base + (NG - g) - 1) // (NG - g)
        pc_slices.append((base, base + take))
        base += take

    state = {}  # per-sample live tiles

    def emit_dma(n):
        xs = xpool.tile([D, HW], mybir.dt.float32r, tag="xs")
        if n == 0:
            # per-group pieces: group-0 matmuls start after ~1/7 of the load
            for grp in groups:
                p0 = CHUNKS[grp[0]][0]
                p1 = CHUNKS[grp[-1]][0] + CHUNKS[grp[-1]][1]
                nc.sync.dma_start(out=xs[:, p0:p1], in_=x_ap[n, :, p0:p1])
        else:
            nc.sync.dma_start(out=xs[:, 0 : HW // 2], in_=x_ap[n, :, 0 : HW // 2])
            nc.sync.dma_start(out=xs[:, HW // 2 :], in_=x_ap[n, :, HW // 2 :])
        state.setdefault(n, {})["xs"] = xs

    def alloc_sample(n):
        st = state.setdefault(n, {})
        # [XT | s] per chunk (bf16): cols 0:128 = X_c^T, col 128 = ||x_p||
        st["xtr"] = xtrpool.tile([128, NCH, 129], bf16, tag="xtr", name="xtr")
        # raw logits [p, c, k]
        st["lgs"] = lpool.tile([128, NCH, K], bf16, tag="lgs", name="lgs")

    def emit_passA_group(n, g):
        st = state[n]
        xs = st["xs"]
        grp = groups[g]
        xt_p = pp_xt.tile([128, GRP, 256], f32, tag="xt")
        for j, c in enumerate(grp):
            p0, w = CHUNKS[c]
            nc.tensor.matmul(
                xt_p[:w, j, 0:256],
                lhsT=xs[:, p0 : p0 + w],
                rhs=identwt[:],
                start=True,
                stop=True,
            )
        return xt_p, grp[0], len(grp)

    def emit_evacs(n, g, xt_p, gc, gn):
        st = state[n]
        xtr, lgs = st["xtr"], st["lgs"]
        # X^T columns -> SBUF bf16 (one consolidated ACT copy per tile)
        nc.scalar.copy(xtr[:, gc : gc + gn, 0:128], xt_p[:, 0:gn, 0:128])
        # logits -> lgs [p, c, k]; two tiles' worth on DVE, rest on ACT
        if g in (1, 3):
            nc.vector.tensor_copy(lgs[:, gc : gc + gn, :], xt_p[:, 0:gn, 128:192])
        else:
            nc.scalar.copy(lgs[:, gc : gc + gn, :], xt_p[:, 0:gn, 128:192])

    # ---- stage B: ss = rowsum(X_c^T^2), inv_s, s-col (sample lag 1) ----
    def st_sq(n):
        st = state[n]
        x2t = scrpool.tile([128, NCH, 128], bf16, tag="x2t", bufs=1)
        st["x2t"] = x2t
        xv = st["xtr"][:, :, 0:128]
        nc.vector.tensor_tensor(out=x2t[:], in0=xv, in1=xv, op=ALU.mult)

    def st_tree12(n):
        st = state[n]
        x2t = st["x2t"]
        t1 = scrpool.tile([128, NCH, 64], bf16, tag="t1", bufs=2)
        t2 = scrpool.tile([128, NCH, 32], bf16, tag="t2", bufs=2)
        st["t2"] = t2
        nc.vector.tensor_tensor(
            out=t1[:], in0=x2t[:, :, 0:64], in1=x2t[:, :, 64:128], op=ALU.add
        )
        nc.vector.tensor_tensor(
            out=t2[:], in0=t1[:, :, 0:32], in1=t1[:, :, 32:64], op=ALU.add
        )

    def st_tree34(n):
        st = state[n]
        t2 = st["t2"]
        t3 = scrpool.tile([128, NCH, 16], bf16, tag="t3", bufs=2)
        t4 = scrpool.tile([128, NCH, 8], bf16, tag="t4", bufs=2)
        ss = smalls.tile([128, NCH], f32, tag="ss")
        st["ss"] = ss
        nc.vector.tensor_tensor(
            out=t3[:], in0=t2[:, :, 0:16], in1=t2[:, :, 16:32], op=ALU.add
        )
        nc.vector.tensor_tensor(
            out=t4[:], in0=t3[:, :, 0:8], in1=t3[:, :, 8:16], op=ALU.add
        )
        nc.vector.tensor_reduce(out=ss[:], in_=t4[:], axis=X_AX, op=ALU.add)

    def st_ischain(n):
        st = state[n]
        lns = smalls.tile([128, NCH], f32, tag="lns")
        isb = smalls.tile([128, NCH], bf16, tag="isb")
        st["isb"] = isb
        nc.scalar.activation(lns[:], st["ss"][:], AF.Ln)
        nc.scalar.activation(isb[:], lns[:], AF.Exp, scale=-0.5)

    def st_scol(n):
        st = state[n]
        # s = ss * inv_s = ||x_p||, into col 128 of each xtr chunk
        nc.gpsimd.tensor_tensor(
            out=st["xtr"][:, :, 128], in0=st["ss"][:], in1=st["isb"][:], op=ALU.mult
        )

    # ---- stage C: scaled logits (sample lag 2, Pool + DVE halves) ----
    def st_slg(n, half, eng):
        st = state[n]
        if "slgt" not in st:
            st["slgt"] = scrpool.tile(
                [128, NCH, K], bf16, tag="slgt", name="slgt", bufs=2
            )
        c0, c1 = (0, NCH // 2) if half == 0 else (NCH // 2, NCH)
        eng.tensor_tensor(
            out=st["slgt"][:, c0:c1, :],
            in0=st["lgs"][:, c0:c1, :],
            in1=bcast(st["isb"][:, c0:c1], K),
            op=ALU.mult,
        )

    # ---- stage D: exp (sample lag 3, ACT) ----
    def st_exp(n, half):
        st = state[n]
        if "et" not in st:
            st["et"] = ebpool.tile([128, NCH, K], bf16, tag="et", name="et")
        c0, c1 = (0, NCH // 2) if half == 0 else (NCH // 2, NCH)
        nc.scalar.activation(
            st["et"][:, c0:c1, :], st["slgt"][:, c0:c1, :], AF.Exp
        )

    # ---- stage E: Z, 1/Z, t = inv_s/Z, sb = e*t (sample lag 4) ----
    def st_zchain(n):
        st = state[n]
        et = st["et"]
        z1 = scrpool.tile([128, NCH, 32], bf16, tag="z1", bufs=2)
        z2 = scrpool.tile([128, NCH, 16], bf16, tag="z2", bufs=2)
        z3 = scrpool.tile([128, NCH, 8], bf16, tag="z3", bufs=2)
        zz = smalls.tile([128, NCH], f32, tag="zz")
        st["zz"] = zz
        nc.vector.tensor_tensor(
            out=z1[:], in0=et[:, :, 0:32], in1=et[:, :, 32:64], op=ALU.add
        )
        nc.vector.tensor_tensor(
            out=z2[:], in0=z1[:, :, 0:16], in1=z1[:, :, 16:32], op=ALU.add
        )
        nc.vector.tensor_tensor(
            out=z3[:], in0=z2[:, :, 0:8], in1=z2[:, :, 8:16], op=ALU.add
        )
        nc.vector.tensor_reduce(out=zz[:], in_=z3[:], axis=X_AX, op=ALU.add)

    def st_recip(n):
        st = state[n]
        rr = smalls.tile([128, NCH], f32, tag="rr")
        st["rr"] = rr
        nc.vector.reciprocal(rr[:], st["zz"][:])

    def st_tsc(n):
        st = state[n]
        tsc = smalls.tile([128, NCH], bf16, tag="tsc")
        st["tsc"] = tsc
        nc.gpsimd.tensor_tensor(
            out=tsc[:], in0=st["isb"][:], in1=st["rr"][:], op=ALU.mult
        )

    def st_sbt(n, half, eng):
        st = state[n]
        if "sbt" not in st:
            st["sbt"] = sbtpool.tile([128, NCH, K], bf16, tag="sbt", name="sbt")
        c0, c1 = (0, NCH // 2) if half == 0 else (NCH // 2, NCH)
        eng.tensor_tensor(
            out=st["sbt"][:, c0:c1, :],
            in0=st["et"][:, c0:c1, :],
            in1=bcast(st["tsc"][:, c0:c1], K),
            op=ALU.mult,
        )

    cstate = {}  # open accumulation tiles for interleaved pass C

    def emit_passC_chunks(n, c0, c1):
        st = state[n]
        xtr, sbt = st["xtr"], st["sbt"]
        if n not in cstate:
            cstate[n] = pp_acc.tile([K, 129], f32, tag="acc", name="acc")
        acc_p = cstate[n]
        for c in range(c0, min(c1, NCH)):
            p0, w = CHUNKS[c]
            nc.tensor.matmul(
                acc_p[:, :],
                lhsT=sbt[:w, c, :],
                rhs=xtr[:w, c, :],
                start=(c == 0),
                stop=(c == NCH - 1),
            )

    def finish_passC(n, agg_all, ssa_all):
        acc_p = cstate.pop(n)
        state.pop(n)
        # evacuate: agg = cols 0:128; sum_sa = col 128
        nc.vector.tensor_copy(agg_all[:, n, :], acc_p[:, 0:D])
        nc.scalar.copy(ssa_all[:, n : n + 1], acc_p[:, 128:129])

    # batched across all samples
    agg_all = tails.tile([K, NS, D], f32)
    ssa_all = tails.tile([K, NS], f32)

    def emit_tail(n0, n1):
        nn = n1 - n0
        agg_h = agg_all[:, n0:n1, :]
        ssa_h = ssa_all[:, n0:n1]
        vl = tails.tile([K, nn, D], f32, tag="t_vl", bufs=2)
        vsq = tails.tile([K, nn * D], f32, tag="t_vsq", bufs=2)
        q = tails.tile([K, nn], f32, tag="t_q", bufs=2)
        qm = tails.tile([K, nn], f32, tag="t_qm", bufs=2)
        isq = tails.tile([K, nn], f32, tag="t_isq", bufs=2)
        isq2 = tails.tile([K, nn], f32, tag="t_isq2", bufs=2)
        u = tails.tile([K, nn], f32, tag="t_u", bufs=2)
        gisr = tails.tile([1, nn], f32, tag="t_gisr", bufs=2)
        gb = tails.tile([K, nn], f32, tag="t_gb", bufs=2)
        sall = tails.tile([K, nn], f32, tag="t_s", bufs=2)
        vf = tails.tile([K, nn, D], f32, tag="t_vf", bufs=2)

        # vl = agg - ssa * cent
        nc.gpsimd.tensor_tensor(
            out=vl[:], in0=bcast(ssa_h, D), in1=mid_bcast(cent_s[:], nn), op=ALU.mult
        )
        nc.vector.tensor_tensor(out=vl[:], in0=agg_h, in1=vl[:], op=ALU.subtract)
        # q = rowsum(vl^2) per (k, n)
        vsqv = vsq[:].rearrange("k (n d) -> k n d", n=nn)
        nc.scalar.activation(vsqv, vl[:], AF.Square)
        nc.vector.tensor_reduce(out=q[:], in_=vsqv, axis=X_AX, op=ALU.add)
        nc.vector.tensor_scalar_max(qm[:], q[:], 1e-24)
        lq = tails.tile([K, nn], f32, tag="t_lq", bufs=2)
        nc.scalar.activation(lq[:], qm[:], AF.Ln)
        nc.scalar.activation(isq[:], lq[:], AF.Exp, scale=-0.5)
        # g = sum_k q_k * isq_k^2  (per sample)
        nc.vector.tensor_tensor(out=isq2[:], in0=isq[:], in1=isq[:], op=ALU.mult)
        nc.vector.tensor_tensor(out=u[:], in0=q[:], in1=isq2[:], op=ALU.mult)
        g_p = pp_tiny.tile([NS, 1], f32, tag="tiny")
        nc.tensor.matmul(
            g_p[:nn, :], lhsT=u[:], rhs=ones_col[:], start=True, stop=True
        )
        # gis = g^-0.5 -> transpose to a row -> broadcast over k partitions
        gm = tails.tile([nn, 1], f32, tag="t_gm", bufs=2)
        nc.vector.tensor_scalar_max(gm[:], g_p[:nn, :], 1e-24)
        gis = tails.tile([nn, 1], f32, tag="t_gis", bufs=2)
        lgm = tails.tile([nn, 1], f32, tag="t_lgm", bufs=2)
        nc.scalar.activation(lgm[:], gm[:], AF.Ln)
        nc.scalar.activation(gis[:], lgm[:], AF.Exp, scale=-0.5)
        gr_p = pp_tiny.tile([1, NS], f32, tag="tiny")
        nc.tensor.matmul(
            gr_p[:, :nn], lhsT=gis[:], rhs=ident[:nn, :nn], start=True, stop=True
        )
        nc.vector.tensor_copy(gisr[:], gr_p[:, :nn])
        gb_p = pp_tiny.tile([K, NS], f32, tag="tiny")
        nc.tensor.matmul(
            gb_p[:, :nn], lhsT=ones_row[:], rhs=gisr[:], start=True, stop=True
        )
        nc.vector.tensor_copy(gb[:], gb_p[:, :nn])
        # s = isq * gb; vf = vl * s
        nc.vector.tensor_tensor(out=sall[:], in0=isq[:], in1=gb[:], op=ALU.mult)
        nc.gpsimd.tensor_tensor(out=vf[:], in0=vl[:], in1=bcast(sall[:], D), op=ALU.mult)
        nc.sync.dma_start(
            out=out_ap.rearrange("n k d -> k n d")[:, n0:n1, :], in_=vf[:]
        )

    # 5-stage software pipeline, one stage lag per round; every stage
    # consumes only tensors finished a full round earlier (no intra-round
    # cross-engine waits):
    #   round r: DMA prefetch(r+1) | passA+evacs(r) | B: ss/inv_s(r-1) |
    #   C: slg(r-2, Pool) | D: exp(r-3, ACT) | E: Z/recip/sb(r-4, DVE) |
    #   passC(r-5) interleaved on PE.
    emit_dma(0)
    emit_dma(1)
    pending_fin = None
    for r in range(NS + 3):
        na = r if r < NS else None
        if na is not None and r + 2 < NS:
            emit_dma(r + 2)
        if na is not None:
            alloc_sample(na)
        nb = r - 1 if 0 <= r - 1 < NS else None
        sd = r - 2 if 0 <= r - 2 < NS else None
        pe = r - 3 if 0 <= r - 3 < NS else None
        # finish the previous round's accumulation at the HEAD of this
        # round's queues so the acc bank frees before passC(pe) needs it
        if pending_fin is not None:
            finish_passC(pending_fin, agg_all, ssa_all)
            if pending_fin % 2 == 1:
                emit_tail(pending_fin - 1, pending_fin + 1)
            pending_fin = None
        for g in range(NG):
            # passC first: it is always ready (sbt a round old), so the PE
            # drains it while a late xs DMA finishes instead of idling
            if pe is not None:
                emit_passC_chunks(pe, *pc_slices[g])
            if na is not None:
                xt_p, gc, gn = emit_passA_group(na, g)
                emit_evacs(na, g, xt_p, gc, gn)
            # keep ACT's queue between consecutive evacuations clear: the
            # PE<->ACT PSUM-recycle loop paces the round, so exp runs late
            # (g4/g5) and the Z/recip/sb chain after the last evacuation
            if g == 0 and nb is not None:
                st_sq(nb)
            elif g == 1 and nb is not None:
                st_tree12(nb)
            elif g == 2 and nb is not None:
                st_tree34(nb)
            elif g == 3 and nb is not None:
                st_ischain(nb)
            elif g == 4:
                if sd is not None:
                    st_exp(sd, 0)
                if nb is not None:
                    fast = na is None or nb >= NS - 2
                    st_slg(nb, 0, nc.vector if fast else nc.gpsimd)
                    st_slg(nb, 1, nc.vector)
            elif g == 5:
                if sd is not None:
                    st_exp(sd, 1)
                if nb is not None:
                    st_scol(nb)
        if sd is not None:
            # round tail: exp(sd) just finished on ACT; chain Z -> 1/Z -> t
            # -> sb here so sb is ready when passC(sd) starts next round
            st_zchain(sd)
            st_recip(sd)
            st_tsc(sd)
            fastb = na is None or sd >= NS - 2
            st_sbt(sd, 0, nc.vector if fastb else nc.gpsimd)
            st_sbt(sd, 1, nc.vector)
        if pe is not None:
            pending_fin = pe
    if pending_fin is not None:
        n_last = pending_fin
        finish_passC(n_last, agg_all, ssa_all)
        emit_tail(n_last - 1, n_last + 1)


def kernel(x, conv_w, centroids):
    from concourse.bass_utils import run_bass_kernel_spmd

    if "nc" not in _CACHE:
        _CACHE["nc"] = _build_nc()
    nc = _CACHE["nc"]

    x = np.ascontiguousarray(np.asarray(x, dtype=np.float32).reshape(N, D, HW))
    wt = np.ascontiguousarray(np.asarray(conv_w, dtype=np.float32).T)
    cent = np.ascontiguousarray(np.asarray(centroids, dtype=np.float32))
    idw = np.zeros((D, 256), dtype=np.float32)
    idw[:, 0:128] = np.eye(D, dtype=np.float32)
    idw[:, 128:192] = wt
    in_maps = [
        {"x": x[i * NS : (i + 1) * NS], "wt": wt, "cent": cent, "idw": idw}
        for i in range(NCORES)
    ]
    res = run_bass_kernel_spmd(nc, in_maps, core_ids=list(range(NCORES))).results
    out = np.concatenate([r["out"].reshape(NS, K * D) for r in res], axis=0)
    return out


if __name__ == "__main__":
    rng = np.random.default_rng(0)
    xs = rng.standard_normal((N, D, 60, 80), dtype=np.float32)
    cw = (rng.standard_normal((K, D)) * 0.1).astype(np.float32)
    ct = rng.random((K, D), dtype=np.float32)
    o = kernel(x=xs, conv_w=cw, centroids=ct)
    print("kernel out", o.shape, o.dtype, np.abs(o).max())

